# revision 62
# baseline (speedup 1.0000x reference)
"""Trainium2 Bass kernel for nn_Attention_37598143710100.

Full attention layer: qkv proj -> rms norm q,k -> rope -> softmax attention
-> out proj.  B=4, L=4096, C=1024, H=16, D=64.

Sharding: 8 cores = (batch b in 0..3) x (query half qh in 0..1).  Each core
computes out[b, qh*2048:(qh+1)*2048, :] completely; host concatenates.
Inside a core the key/value positions are permuted to [own-half | other-half]
so the SPMD program is identical across cores (softmax is order-invariant).

v4: one continuous ACT-bound pipeline.  ScalarE exp (1024 x ~1.15us) is the
algorithmic floor.  Attention blocks process 512 queries x 2 heads: each
score PSUM tile [128,1024] holds head0|head1 halves written by two
row-packed matmuls that run CONCURRENTLY in the PE array (rows 0-63 /
64-127), one exp covers both, and the two ctx accumulators are [65,512]
(1 PSUM bank each).  PSUM: scores 2x2 + ctx 2x1 + proj 2 = 8 banks, so the
NEXT head-pair's qkv projection (and the tail of the output projection)
drips into the attention instruction stream and fully overlaps.
"""

import numpy as np
import ml_dtypes

B, L, C, H, D = 4, 4096, 1024, 16, 64
NCORES = 8
LQ = L // 2
EPS = 1e-6
NPAIR = H // 2
RSQ_A, RSQ_B = 1.3750, 0.2700  # rsqrt Newton init y0 = A - B*x on [0.4, 3.5]

_compiled = None


def _build():
    import concourse.tile as tile
    from concourse import bacc, mybir
    from concourse.masks import make_identity

    bf16 = mybir.dt.bfloat16
    f32 = mybir.dt.float32
    AF = mybir.ActivationFunctionType

    nc = bacc.Bacc("TRN2", target_bir_lowering=False, debug=False,
                   enable_asserts=True, num_devices=NCORES)

    xT = nc.dram_tensor("xT", [C, L], bf16, kind="ExternalInput").ap()
    wT = nc.dram_tensor("wT", [C, 3 * C], bf16, kind="ExternalInput").ap()
    wpT = nc.dram_tensor("wpT", [C, C], bf16, kind="ExternalInput").ap()
    cgq = nc.dram_tensor("cgq", [D, LQ], bf16, kind="ExternalInput").ap()
    sgq = nc.dram_tensor("sgq", [D, LQ], bf16, kind="ExternalInput").ap()
    cgk = nc.dram_tensor("cgk", [D, L], bf16, kind="ExternalInput").ap()
    sgk = nc.dram_tensor("sgk", [D, L], bf16, kind="ExternalInput").ap()
    onesAB = nc.dram_tensor("onesAB", [128, 2], bf16, kind="ExternalInput").ap()
    bproj = nc.dram_tensor("bproj", [1, C], f32, kind="ExternalInput").ap()
    out_ap = nc.dram_tensor("out", [LQ, C], f32, kind="ExternalOutput").ap()

    # scratch: rms inv rows (q: 2jc+h, k: 8+2jc+h), softmax recips, ctx bounce
    inv_sc = nc.dram_tensor("inv_sc", [NPAIR, 24, 512], bf16).ap()
    rcp_sc = nc.dram_tensor("rcp_sc", [NPAIR, 4, 2, 512], bf16).ap()
    ctx_sc = nc.dram_tensor("ctx_sc", [NPAIR, 128, LQ], bf16).ap()
    # k/v exchange between sibling cores (same batch, other query half):
    # each core computes khat+vsb for its OWN 2048 keys, AllGathers within
    # the pair, and reads back both halves.  Softmax is key-order-invariant,
    # so the rank-ordered gather layout needs no per-core fixup.
    kex_in = nc.dram_tensor("kex_in", [NPAIR, 128, 4128], bf16).ap()
    kex_out = nc.dram_tensor("kex_out", [NPAIR, 2, 128, 4128], bf16).ap()

    xTr = xT.rearrange("(eb p) j -> p eb j", p=128)
    wTr = wT.rearrange("(eb p) f -> p eb f", p=128)
    wpTr = wpT.rearrange("(cb p) o -> p cb o", p=128)

    with tile.TileContext(nc) as tc:
        with tc.tile_pool(name="persist", bufs=1) as persist, \
             tc.tile_pool(name="pairq", bufs=2) as pairq, \
             tc.tile_pool(name="pairk", bufs=2) as pairk, \
             tc.tile_pool(name="pairv", bufs=2) as pairv, \
             tc.tile_pool(name="trans", bufs=1) as trans, \
             tc.tile_pool(name="wslp", bufs=2) as wslp, \
             tc.tile_pool(name="work", bufs=2) as work, \
             tc.tile_pool(name="nwt", bufs=1) as nwt, \
             tc.tile_pool(name="drn", bufs=1) as drn, \
             tc.tile_pool(name="xe", bufs=16) as xep, \
             tc.tile_pool(name="exps", bufs=4) as exps, \
             tc.tile_pool(name="outp", bufs=2) as outp:

            ident = persist.tile([128, 128], bf16, tag="ident")
            make_identity(nc, ident[:])
            onesT = persist.tile([128, 2], bf16, tag="onesT")
            nc.sync.dma_start(onesT[:], onesAB[:])

            # allocated here, loaded after hp0's weight/x DMAs are in flight
            cgq_b = persist.tile([128, LQ], bf16, tag="cgq")
            sgq_b = persist.tile([128, LQ], bf16, tag="sgq")
            # k tables only for the core's OWN keys (first LQ of the perm)
            cgk_b = persist.tile([128, LQ], bf16, tag="cgk")
            sgk_b = persist.tile([128, LQ], bf16, tag="sgk")
            wp_sb = persist.tile([128, 8, C], bf16, tag="wp")
            bp_b = persist.tile([128, C], f32, tag="bp")

            def load_persist():
                for t, src in ((cgk_b, cgk[:, 0:LQ]), (sgk_b, sgk[:, 0:LQ]),
                               (cgq_b, cgq), (sgq_b, sgq)):
                    nc.sync.dma_start(t[0:64, :], src)
                    nc.sync.dma_start(t[64:128, :], src)
                nc.sync.dma_start(wp_sb[:], wpTr[:])
                nc.sync.dma_start(bp_b[:],
                                  bproj[0:1, :].partition_broadcast(128))

            with tc.tile_pool(name="scp", bufs=2, space="PSUM") as scp, \
                 tc.tile_pool(name="cxp", bufs=2, space="PSUM") as cxp, \
                 tc.tile_pool(name="prj", bufs=2, space="PSUM") as prj:

                hp_state = {}

                # ============== projection steps (dripped) ==============
                def build_proj_steps(hp, per_jc_inv=False):
                    """Closures, each issuing a small slice of head-pair
                    hp's qkv projection; executed interleaved into the
                    previous head-pair's attention instruction stream.
                    per_jc_inv: compute the rms-inv + rope per jc chunk
                    (more DVE ops, but each k/q chunk is ready immediately
                    -- used for hp0 so its attention starts early)."""
                    st = {}
                    hp_state[hp] = st

                    def s_alloc():
                        w = wslp.tile([128, 8, 3, 128], bf16, tag="wsl",
                                      name=f"wsl{hp}")
                        for t in range(3):
                            nc.sync.dma_start(
                                w[:, :, t, :],
                                wTr[:, :, t * C + hp * 128:
                                    t * C + (hp + 1) * 128])
                        st["w"] = w
                        st["qraw"] = trans.tile([128, LQ], bf16, tag="qraw",
                                                name=f"qraw{hp}")
                        st["kraw"] = trans.tile([128, LQ], bf16, tag="kraw",
                                                name=f"kraw{hp}")
                        st["qshf"] = trans.tile([128, LQ], bf16, tag="qshf",
                                                name=f"qshf{hp}")
                        st["kshf"] = trans.tile([128, LQ], bf16, tag="kshf",
                                                name=f"kshf{hp}")
                        st["coll"] = trans.tile([16, 512], bf16, tag="coll",
                                                name=f"coll{hp}")
                        st["collb"] = trans.tile([8, 512], bf16, tag="collb",
                                                 name=f"collb{hp}")
                        st["qhat"] = pairq.tile([128, LQ], bf16, tag="qhat",
                                                name=f"qhat{hp}")
                        st["khat"] = pairk.tile([128, L], bf16, tag="khat",
                                                name=f"khat{hp}")
                        st["vsb"] = pairv.tile([128, 32, 2, 65], bf16,
                                               tag="vsb", name=f"vsb{hp}")
                        nc.vector.memset(st["vsb"][:, :, :, 64:65], 1.0)
                        st["xe"] = {}
                        for eb in range(8):
                            prefetch(0, eb)
                    ALLOC = [s_alloc]

                    def prefetch(jc, eb):
                        t = xep.tile([128, 512], bf16, tag="xe", name="xe")
                        nc.gpsimd.dma_start(t[:], xTr[:, eb, jc * 512:
                                                      (jc + 1) * 512])
                        st["xe"][(jc, eb)] = t

                    def palloc(shape, dtype, name):
                        # during hp0's serial head the attention score pool
                        # is idle -- borrow its banks for a 4-deep
                        # accumulator rotation (single monotone stream, so
                        # no cross-stream pool-order inversion)
                        if st.get("wide"):
                            st["pc"] = st.get("pc", 0) + 1
                            if st["pc"] % 2:
                                return scp.tile(shape, dtype, tag="sc",
                                                name=name)
                        return prj.tile(shape, dtype, tag="pj", name=name)

                    def mk_mm(jc, which, eb, pf):
                        def f():
                            if eb == 0:
                                st["ps"] = palloc([128, 512], f32, "ps")
                            if pf is not None:
                                prefetch(*pf)
                            nc.tensor.matmul(st["ps"][:],
                                             st["w"][:, eb, which, :],
                                             st["xe"][(jc, eb)][:],
                                             start=(eb == 0), stop=(eb == 7))
                        return f

                    def mk_sum(jc, tsr):
                        # rms partial sums + rotate-half shifted copy
                        def f():
                            raw = st["qraw"] if tsr == 0 else st["kraw"]
                            shf = st["qshf"] if tsr == 0 else st["kshf"]
                            sl = slice(jc * 512, (jc + 1) * 512)
                            nc.vector.tensor_copy(raw[:, sl], st["ps"][:])
                            sq = work.tile([128, 512], bf16, tag="sq",
                                           name="sq")
                            nc.vector.tensor_mul(sq[:], raw[:, sl], raw[:, sl])
                            pss = palloc([2, 512], f32, "pss")
                            nc.tensor.matmul(pss[:], onesT[:], sq[:],
                                             start=True, stop=True)
                            cp2 = work.tile([2, 512], bf16, tag="cp2",
                                            name="cp2")
                            nc.vector.tensor_copy(cp2[:], pss[:])
                            if per_jc_inv:
                                st["cp2"] = cp2
                            elif tsr == 0:
                                nc.sync.dma_start(
                                    st["coll"][2 * jc:2 * jc + 2, :], cp2[:])
                            elif jc < 4:
                                nc.sync.dma_start(
                                    st["coll"][8 + 2 * jc:10 + 2 * jc, :],
                                    cp2[:])
                            else:
                                nc.sync.dma_start(
                                    st["collb"][2 * (jc - 4):
                                                2 * (jc - 4) + 2, :], cp2[:])
                            nc.sync.dma_start(shf[0:32, sl], raw[32:64, sl])
                            nc.sync.dma_start(shf[32:64, sl], raw[0:32, sl])
                            nc.sync.dma_start(shf[64:96, sl], raw[96:128, sl])
                            nc.sync.dma_start(shf[96:128, sl], raw[64:96, sl])
                        return f

                    def mk_vcopy(jc):
                        def f():
                            vTc = work.tile([128, 512], bf16, tag="vTc",
                                            name="vTc")
                            nc.vector.tensor_copy(vTc[:], st["ps"][:])
                            st["vTc"] = vTc
                        return f

                    def mk_vtr(jc, jt):
                        def f():
                            jg = jc * 4 + jt
                            pt = palloc([128, 128], bf16, "pt")
                            nc.tensor.transpose(
                                pt[:], st["vTc"][:, jt * 128:(jt + 1) * 128],
                                ident[:])
                            nc.vector.tensor_copy(
                                st["vsb"][:, jg, :, 0:64],
                                pt[:].rearrange("p (h d) -> p h d", h=2))
                        return f

                    def mk_newton2(jc, tsr):
                        # per-chunk rsqrt Newton on [2,512] straight from
                        # the fresh cp2 sums; writes its two inv_sc rows
                        def f():
                            src = st["cp2"]
                            xms = nwt.tile([2, 512], f32, tag="xms",
                                           name="n2x")
                            nc.vector.tensor_scalar(
                                xms[:], src[:], 1.0 / 64.0, EPS,
                                op0=mybir.AluOpType.mult,
                                op1=mybir.AluOpType.add)
                            y = nwt.tile([2, 512], f32, tag="y", name="n2y")
                            nc.vector.tensor_scalar(
                                y[:], xms[:], -RSQ_B, RSQ_A,
                                op0=mybir.AluOpType.mult,
                                op1=mybir.AluOpType.add)
                            t1 = nwt.tile([2, 512], f32, tag="t1",
                                          name="n2t1")
                            t2 = nwt.tile([2, 512], f32, tag="t2",
                                          name="n2t2")
                            inv = nwt.tile([2, 512], bf16, tag="inv",
                                           name="n2i")
                            for it in range(3):
                                nc.vector.tensor_mul(t1[:], xms[:], y[:])
                                nc.vector.tensor_mul(t2[:], t1[:], y[:])
                                nc.vector.tensor_scalar(
                                    t2[:], t2[:], -0.5, 1.5,
                                    op0=mybir.AluOpType.mult,
                                    op1=mybir.AluOpType.add)
                                if it < 2:
                                    nc.vector.tensor_mul(y[:], y[:], t2[:])
                                else:
                                    nc.vector.tensor_mul(inv[:], y[:], t2[:])
                            r0 = 2 * jc if tsr == 0 else 8 + 2 * jc
                            nc.sync.dma_start(inv_sc[hp, r0:r0 + 2, :],
                                              inv[:])
                        return f

                    def jc_steps(jc):
                        s = []
                        for eb in range(8):
                            # prefetch next jc's x a full pass ahead
                            pf = (jc + 1, eb) if jc < 3 else None
                            s.append(mk_mm(jc, 1, eb, pf))
                        s.append(mk_sum(jc, 1))
                        if per_jc_inv:
                            s.append(mk_newton2(jc, 1))
                            s.append(mk_rope(jc, 1))
                        for eb in range(8):
                            s.append(mk_mm(jc, 2, eb, None))
                        s.append(mk_vcopy(jc))
                        for jt in range(4):
                            s.append(mk_vtr(jc, jt))
                        if jc < 4:
                            for eb in range(8):
                                s.append(mk_mm(jc, 0, eb, None))
                            s.append(mk_sum(jc, 0))
                            if per_jc_inv:
                                s.append(mk_newton2(jc, 0))
                                s.append(mk_rope(jc, 0))
                        return s

                    def mk_newton(seg):
                        # rsqrt(ms+eps) via Newton on DVE; seg 0 covers
                        # inv rows 0..15 (q + first-half k), seg 1 rows 16..23
                        def f():
                            src = st["coll"] if seg == 0 else st["collb"]
                            nr = 16 if seg == 0 else 8
                            lo = 0 if seg == 0 else 16
                            xms = nwt.tile([16, 512], f32, tag="xms",
                                           name="xms")[0:nr, :]
                            nc.vector.tensor_scalar(
                                xms[:], src[:], 1.0 / 64.0, EPS,
                                op0=mybir.AluOpType.mult,
                                op1=mybir.AluOpType.add)
                            y = nwt.tile([16, 512], f32, tag="y",
                                         name="y")[0:nr, :]
                            nc.vector.tensor_scalar(
                                y[:], xms[:], -RSQ_B, RSQ_A,
                                op0=mybir.AluOpType.mult,
                                op1=mybir.AluOpType.add)
                            t1 = nwt.tile([16, 512], f32, tag="t1",
                                          name="t1")[0:nr, :]
                            t2 = nwt.tile([16, 512], f32, tag="t2",
                                          name="t2")[0:nr, :]
                            inv = nwt.tile([16, 512], bf16, tag="inv",
                                           name="inv")[0:nr, :]
                            for it in range(3):
                                nc.vector.tensor_mul(t1[:], xms[:], y[:])
                                nc.vector.tensor_mul(t2[:], t1[:], y[:])
                                nc.vector.tensor_scalar(
                                    t2[:], t2[:], -0.5, 1.5,
                                    op0=mybir.AluOpType.mult,
                                    op1=mybir.AluOpType.add)
                                if it < 2:
                                    nc.vector.tensor_mul(y[:], y[:], t2[:])
                                else:
                                    nc.vector.tensor_mul(inv[:], y[:], t2[:])
                            nc.sync.dma_start(inv_sc[hp, lo:lo + nr, :],
                                              inv[:])
                        return f

                    def mk_rope(jc, tsr):
                        # hat = raw*inv*cg + shf*inv*sg  for one 512-chunk
                        def f():
                            if tsr == 0:
                                raw, shf, cg_b, sg_b = (st["qraw"], st["qshf"],
                                                        cgq_b, sgq_b)
                                hat, r0 = st["qhat"], 0
                            else:
                                raw, shf, cg_b, sg_b = (st["kraw"], st["kshf"],
                                                        cgk_b, sgk_b)
                                hat, r0 = st["khat"], 8
                            sl = slice(jc * 512, (jc + 1) * 512)
                            rA = r0 + 2 * jc
                            ib = work.tile([128, 512], bf16, tag="ib",
                                           name="ib")
                            nc.sync.dma_start(
                                ib[0:64, :],
                                inv_sc[hp, rA:rA + 1, :]
                                .partition_broadcast(64))
                            nc.sync.dma_start(
                                ib[64:128, :],
                                inv_sc[hp, rA + 1:rA + 2, :]
                                .partition_broadcast(64))
                            icg = work.tile([128, 512], bf16, tag="icg",
                                            name="icg")
                            nc.vector.tensor_mul(icg[:], ib[:], cg_b[:, sl])
                            isg = work.tile([128, 512], bf16, tag="isg",
                                            name="isg")
                            nc.vector.tensor_mul(isg[:], ib[:], sg_b[:, sl])
                            u = work.tile([128, 512], bf16, tag="u", name="u")
                            nc.vector.tensor_mul(u[:], raw[:, sl], icg[:])
                            v2 = work.tile([128, 512], bf16, tag="v2",
                                           name="v2")
                            nc.vector.tensor_mul(v2[:], shf[:, sl], isg[:])
                            nc.vector.tensor_add(hat[:, sl], u[:], v2[:])
                        return f

                    def mk_exchange():
                        # own khat/vsb halves -> DRAM, AllGather within the
                        # core pair, read back [slot0|slot1].  Odd cores see
                        # keys as [sibling|own] -- softmax is key-order
                        # invariant, so no per-core fixup is needed.
                        def send():
                            nc.sync.dma_start(kex_in[hp, :, 0:LQ],
                                              st["khat"][:, 0:LQ])
                            nc.sync.dma_start(
                                kex_in[hp, :, LQ:4128].rearrange(
                                    "p (j h d) -> p j h d", j=16, h=2),
                                st["vsb"][:, 0:16, :, :])

                        def gather():
                            nc.gpsimd.collective_compute(
                                "AllGather", mybir.AluOpType.bypass,
                                replica_groups=[[0, 1], [2, 3], [4, 5],
                                                [6, 7]],
                                ins=[kex_in[hp, :, :]],
                                outs=[kex_out[hp, :, :, :]])

                        def recv():
                            for s in range(2):
                                nc.sync.dma_start(
                                    st["khat"][:, s * LQ:(s + 1) * LQ],
                                    kex_out[hp, s, :, 0:LQ])
                                nc.sync.dma_start(
                                    st["vsb"][:, s * 16:(s + 1) * 16, :, :],
                                    kex_out[hp, s, :, LQ:4128].rearrange(
                                        "p (j h d) -> p j h d", j=16, h=2))
                        return [send, gather, recv]

                    # own-keys only: k/v/q for jc0-3, one newton batch,
                    # ropes, then the pair exchange
                    steps = list(ALLOC)
                    for jc in range(4):
                        steps += jc_steps(jc)
                    steps.append(mk_newton(0))
                    for jc in range(4):
                        steps.append(mk_rope(jc, 0))
                        steps.append(mk_rope(jc, 1))
                    steps += mk_exchange()
                    return steps

                # ============== output projection steps ==============
                def build_outproj_steps(ib_lo, ib_hi):
                    steps = []
                    st = {}

                    def mk_pref(ib):
                        def f():
                            cts = []
                            for cb in range(8):
                                ct = outp.tile([128, 128], bf16, tag="ct",
                                               name="ct", bufs=18)
                                nc.sync.dma_start(
                                    ct[:],
                                    ctx_sc[cb, :, ib * 128:(ib + 1) * 128])
                                cts.append(ct)
                            st[ib] = cts
                        return f

                    def mk_opmm(ib, half, cb):
                        def f():
                            if cb == 0:
                                st["po"] = prj.tile([128, 512], f32, tag="pj",
                                                    name="po")
                            nc.tensor.matmul(
                                st["po"][:], st[ib][cb][:],
                                wp_sb[:, cb, half * 512:(half + 1) * 512],
                                start=(cb == 0), stop=(cb == 7))
                        return f

                    def mk_epi(ib, half):
                        def f():
                            ot = outp.tile([128, 512], f32, tag="ot",
                                           name="ot")
                            nc.vector.tensor_add(
                                ot[:], st["po"][:],
                                bp_b[:, half * 512:(half + 1) * 512])
                            nc.sync.dma_start(
                                out_ap[ib * 128:(ib + 1) * 128,
                                       half * 512:(half + 1) * 512], ot[:])
                        return f

                    def mm_steps(ib):
                        s = []
                        for half in range(2):
                            for cb in range(8):
                                s.append(mk_opmm(ib, half, cb))
                            s.append(mk_epi(ib, half))
                        return s

                    prev = None
                    for ib in range(ib_lo, ib_hi):
                        steps.append(mk_pref(ib))
                        if prev is not None:
                            steps += mm_steps(prev)
                        prev = ib
                    steps += mm_steps(prev)
                    return steps

                class Dripper:
                    """Issues `steps` gradually over slots [lo, hi).  Multiple
                    drippers on one attention pass MUST have disjoint windows:
                    interleaving two projection streams rotates the 2-slot
                    PSUM/xe pools out of program order and deadlocks the
                    in-order engine queues."""

                    def __init__(self, steps, lo, hi):
                        self.steps = steps
                        self.lo = lo
                        self.hi = hi
                        self.i = 0

                    def drip(self, slot):
                        if slot < self.lo:
                            return
                        frac = (slot + 1 - self.lo) / (self.hi - self.lo)
                        tgt = min(len(self.steps),
                                  int(frac * len(self.steps)) + 1)
                        while self.i < tgt:
                            self.steps[self.i]()
                            self.i += 1

                    def finish(self):
                        while self.i < len(self.steps):
                            self.steps[self.i]()
                            self.i += 1

                # =================== attention (flat) ===================
                # 128 iterations per head-pair: 4 query-chunks x 32 key-
                # chunks.  Scores are issued 2 iterations ahead -- across
                # chunk AND head-pair boundaries -- so ACT never waits.
                def issue_sc(g):
                    hp, r = divmod(g, 128)
                    qc, j = divmod(r, 32)
                    st = hp_state[hp]
                    khp, qhp = st["khat"], st["qhat"]
                    q0 = qc * 512
                    sct = scp.tile([128, 1024], f32, tag="sc", name="sct")
                    for h in range(2):
                        nc.tensor.matmul(
                            sct[:, h * 512:(h + 1) * 512],
                            khp[h * 64:(h + 1) * 64, j * 128:(j + 1) * 128],
                            qhp[h * 64:(h + 1) * 64, q0:q0 + 512],
                            start=True, stop=True,
                            tile_position=(h * 64, 0))
                    return sct

                def drain(hp, qc, ctxp):
                    # scale rows 0..63 by 1/row64, bounce to DRAM
                    q0 = qc * 512
                    ctf = [drn.tile([65, 512], bf16, tag=f"ctf{h}",
                                    name=f"ctf{h}") for h in range(2)]
                    for h in range(2):
                        nc.vector.tensor_copy(ctf[h][:], ctxp[h][:])
                    for h in range(2):
                        rcs = drn.tile([1, 512], f32, tag=f"rcs{h}",
                                       name=f"rcs{h}")
                        nc.vector.tensor_copy(rcs[:], ctf[h][64:65, :])
                        rcp = drn.tile([1, 512], f32, tag=f"rcp{h}",
                                       name=f"rcp{h}")
                        nc.vector.reciprocal_approx_fast(out=rcp[:],
                                                         in_=rcs[:])
                        rcpb = drn.tile([1, 512], bf16, tag=f"rcpb{h}",
                                        name=f"rcpb{h}")
                        nc.vector.tensor_copy(rcpb[:], rcp[:])
                        nc.sync.dma_start(rcp_sc[hp, qc, h:h + 1, :],
                                          rcpb[:])
                    for h in range(2):
                        rb = drn.tile([64, 512], bf16, tag=f"rb{h}",
                                      name=f"rb{h}")
                        nc.sync.dma_start(
                            rb[:],
                            rcp_sc[hp, qc, h:h + 1, :].partition_broadcast(64))
                        cto = drn.tile([64, 512], bf16, tag=f"cto{h}",
                                       name=f"cto{h}")
                        nc.vector.tensor_mul(cto[:], ctf[h][0:64, :], rb[:])
                        nc.sync.dma_start(
                            ctx_sc[hp, h * 64:(h + 1) * 64, q0:q0 + 512],
                            cto[:])

                def attention_hp(hp, drippers, pre):
                    # pre: dict g -> sc tile issued by the previous pair
                    st = hp_state[hp]
                    vhp = st["vsb"]
                    g0 = hp * 128
                    sc_t = dict(pre)
                    carry = {}
                    ctxp = None
                    for r in range(128):
                        g = g0 + r
                        qc, j = divmod(r, 32)
                        if j == 0:
                            ctxp = [cxp.tile([65, 512], f32, tag="cx",
                                             name=f"cx{h}")
                                    for h in range(2)]
                        if g not in sc_t:
                            sc_t[g] = issue_sc(g)
                        e = exps.tile([128, 1024], bf16, tag="exps", name="e")
                        nc.scalar.activation(e[:], sc_t.pop(g)[:], AF.Exp,
                                             scale=0.125)
                        for ga in (g + 1, g + 2):
                            if ga in sc_t or ga >= 1024:
                                continue
                            if ga < g0 + 128:
                                sc_t[ga] = issue_sc(ga)
                            else:
                                carry[ga] = issue_sc(ga)
                        for d in drippers:
                            d.drip(r)
                        for h in range(2):
                            nc.tensor.matmul(
                                ctxp[h][:], vhp[:, j, h, :],
                                e[:, h * 512:(h + 1) * 512],
                                start=(j == 0), stop=(j == 31))
                        if j == 31:
                            drain(hp, qc, ctxp)
                    for d in drippers:
                        d.finish()
                    return carry

                # ====================== main loop =======================
                steps0 = build_proj_steps(0)
                hp_state[0]["wide"] = True
                steps0[0]()  # hp0 weight + x DMAs first in the queues
                load_persist()
                for s in steps0[1:]:
                    s()
                hp_state[0]["wide"] = False
                carry = {}
                for hp in range(NPAIR):
                    drippers = []
                    if hp + 1 < NPAIR:
                        # finish by slot 110: the pair-exchange collective at
                        # the stream tail needs link time before the next
                        # head-pair's attention reads khat/vsb
                        drippers.append(
                            Dripper(build_proj_steps(hp + 1), 0, 110))
                    else:
                        # last pair: drip the out projection for each query
                        # chunk right after that chunk's ctx drains
                        drippers.append(
                            Dripper(build_outproj_steps(0, 4), 33, 62))
                        drippers.append(
                            Dripper(build_outproj_steps(4, 8), 65, 94))
                        drippers.append(
                            Dripper(build_outproj_steps(8, 12), 97, 126))
                    carry = attention_hp(hp, drippers, carry)

                # remaining output projection (queries 1536..2047)
                for s in build_outproj_steps(12, 16):
                    s()

    nc.compile()
    return nc


def _host_prep(x, W_qkv, q_scale, k_scale, W_proj, b_proj, cos, sin):
    nbf = ml_dtypes.bfloat16
    cosn = np.asarray(cos, np.float32)
    sinn = np.asarray(sin, np.float32)
    qs = np.asarray(q_scale, np.float32)
    ks = np.asarray(k_scale, np.float32)

    def tables(g):
        sign = np.concatenate([-np.ones(D // 2), np.ones(D // 2)]).astype(np.float32)
        gpart = np.concatenate([g[D // 2:], g[:D // 2]])
        cg = cosn * g[None, :]
        sg = sinn * (sign * gpart)[None, :]
        return cg.T.copy(), sg.T.copy()

    cgq_f, sgq_f = tables(qs)
    cgk_f, sgk_f = tables(ks)

    wT = np.asarray(W_qkv, np.float32).T.astype(nbf)
    wpT = np.asarray(W_proj, np.float32).T.astype(nbf)
    bp = np.asarray(b_proj, np.float32).reshape(1, C)
    onesAB = np.zeros((128, 2), nbf)
    onesAB[0:64, 0] = 1.0
    onesAB[64:128, 1] = 1.0

    xn = np.asarray(x, np.float32)
    in_maps = []
    for core in range(NCORES):
        b, qh = core // 2, core % 2
        own = slice(qh * LQ, (qh + 1) * LQ)
        perm = np.r_[np.arange(qh * LQ, (qh + 1) * LQ),
                     np.arange((1 - qh) * LQ, (2 - qh) * LQ)]
        xTc = xn[b].T[:, perm].astype(nbf)
        in_maps.append({
            "xT": np.ascontiguousarray(xTc),
            "wT": wT, "wpT": wpT,
            "cgq": np.ascontiguousarray(cgq_f[:, own]).astype(nbf),
            "sgq": np.ascontiguousarray(sgq_f[:, own]).astype(nbf),
            "cgk": np.ascontiguousarray(cgk_f[:, perm]).astype(nbf),
            "sgk": np.ascontiguousarray(sgk_f[:, perm]).astype(nbf),
            "onesAB": onesAB, "bproj": bp,
        })
    return in_maps


def kernel(x, W_qkv, q_scale, k_scale, W_proj, b_proj, cos, sin, _trace=False):
    global _compiled
    from concourse.bass_utils import run_bass_kernel_spmd
    if _compiled is None:
        _compiled = _build()
    in_maps = _host_prep(x, W_qkv, q_scale, k_scale, W_proj, b_proj, cos, sin)
    res = run_bass_kernel_spmd(_compiled, in_maps, core_ids=list(range(NCORES)),
                               trace=_trace)
    out = np.empty((B, L, C), np.float32)
    for core in range(NCORES):
        b, qh = core // 2, core % 2
        out[b, qh * LQ:(qh + 1) * LQ, :] = res.results[core]["out"]
    kernel._last = res
    return out


# revision 64
# speedup vs baseline: 1.0331x; 1.0331x over previous
"""Trainium2 Bass kernel for nn_Attention_37598143710100.

Full attention layer: qkv proj -> rms norm q,k -> rope -> softmax attention
-> out proj.  B=4, L=4096, C=1024, H=16, D=64.

Sharding: 8 cores = (batch b in 0..3) x (query half qh in 0..1).  Each core
computes out[b, qh*2048:(qh+1)*2048, :] completely; host concatenates.
Inside a core the key/value positions are permuted to [own-half | other-half]
so the SPMD program is identical across cores (softmax is order-invariant).

v4: one continuous ACT-bound pipeline.  ScalarE exp (1024 x ~1.15us) is the
algorithmic floor.  Attention blocks process 512 queries x 2 heads: each
score PSUM tile [128,1024] holds head0|head1 halves written by two
row-packed matmuls that run CONCURRENTLY in the PE array (rows 0-63 /
64-127), one exp covers both, and the two ctx accumulators are [65,512]
(1 PSUM bank each).  PSUM: scores 2x2 + ctx 2x1 + proj 2 = 8 banks, so the
NEXT head-pair's qkv projection (and the tail of the output projection)
drips into the attention instruction stream and fully overlaps.
"""

import numpy as np
import ml_dtypes

B, L, C, H, D = 4, 4096, 1024, 16, 64
NCORES = 8
LQ = L // 2
EPS = 1e-6
NPAIR = H // 2
RSQ_A, RSQ_B = 1.3750, 0.2700  # rsqrt Newton init y0 = A - B*x on [0.4, 3.5]

_compiled = None


def _build():
    import concourse.tile as tile
    from concourse import bacc, mybir
    from concourse.masks import make_identity

    bf16 = mybir.dt.bfloat16
    f32 = mybir.dt.float32
    AF = mybir.ActivationFunctionType

    nc = bacc.Bacc("TRN2", target_bir_lowering=False, debug=False,
                   enable_asserts=True, num_devices=NCORES)

    xT = nc.dram_tensor("xT", [C, L], bf16, kind="ExternalInput").ap()
    wT = nc.dram_tensor("wT", [C, 3 * C], bf16, kind="ExternalInput").ap()
    wpT = nc.dram_tensor("wpT", [C, C], bf16, kind="ExternalInput").ap()
    cgq = nc.dram_tensor("cgq", [D, LQ], bf16, kind="ExternalInput").ap()
    sgq = nc.dram_tensor("sgq", [D, LQ], bf16, kind="ExternalInput").ap()
    cgk = nc.dram_tensor("cgk", [D, L], bf16, kind="ExternalInput").ap()
    sgk = nc.dram_tensor("sgk", [D, L], bf16, kind="ExternalInput").ap()
    onesAB = nc.dram_tensor("onesAB", [128, 2], bf16, kind="ExternalInput").ap()
    bproj = nc.dram_tensor("bproj", [1, C], f32, kind="ExternalInput").ap()
    out_ap = nc.dram_tensor("out", [LQ, C], f32, kind="ExternalOutput").ap()

    # scratch: rms inv rows (q: 2jc+h, k: 8+2jc+h), softmax recips, ctx bounce
    inv_sc = nc.dram_tensor("inv_sc", [NPAIR, 24, 512], bf16).ap()
    rcp_sc = nc.dram_tensor("rcp_sc", [NPAIR, 4, 2, 512], bf16).ap()
    ctx_sc = nc.dram_tensor("ctx_sc", [NPAIR, 128, LQ], bf16).ap()
    # k/v exchange between sibling cores (same batch, other query half):
    # each core computes khat+vsb for its OWN 2048 keys, AllGathers within
    # the pair, and reads back both halves.  Softmax is key-order-invariant,
    # so the rank-ordered gather layout needs no per-core fixup.
    kex_in = nc.dram_tensor("kex_in", [NPAIR, 128, 4128], bf16).ap()
    kex_out = nc.dram_tensor("kex_out", [NPAIR, 2, 128, 4128], bf16).ap()

    xTr = xT.rearrange("(eb p) j -> p eb j", p=128)
    wTr = wT.rearrange("(eb p) f -> p eb f", p=128)
    wpTr = wpT.rearrange("(cb p) o -> p cb o", p=128)

    with tile.TileContext(nc) as tc:
        with tc.tile_pool(name="persist", bufs=1) as persist, \
             tc.tile_pool(name="pairq", bufs=2) as pairq, \
             tc.tile_pool(name="pairk", bufs=2) as pairk, \
             tc.tile_pool(name="pairv", bufs=2) as pairv, \
             tc.tile_pool(name="trans", bufs=1) as trans, \
             tc.tile_pool(name="wslp", bufs=2) as wslp, \
             tc.tile_pool(name="work", bufs=2) as work, \
             tc.tile_pool(name="nwt", bufs=1) as nwt, \
             tc.tile_pool(name="drn", bufs=1) as drn, \
             tc.tile_pool(name="xe", bufs=16) as xep, \
             tc.tile_pool(name="exps", bufs=4) as exps, \
             tc.tile_pool(name="outp", bufs=2) as outp:

            ident = persist.tile([128, 128], bf16, tag="ident")
            make_identity(nc, ident[:])
            onesT = persist.tile([128, 2], bf16, tag="onesT")
            nc.sync.dma_start(onesT[:], onesAB[:])

            # allocated here, loaded after hp0's weight/x DMAs are in flight
            cgq_b = persist.tile([128, LQ], bf16, tag="cgq")
            sgq_b = persist.tile([128, LQ], bf16, tag="sgq")
            # k tables only for the core's OWN keys (first LQ of the perm)
            cgk_b = persist.tile([128, LQ], bf16, tag="cgk")
            sgk_b = persist.tile([128, LQ], bf16, tag="sgk")
            wp_sb = persist.tile([128, 8, C], bf16, tag="wp")
            bp_b = persist.tile([128, C], f32, tag="bp")

            def load_persist():
                for t, src in ((cgk_b, cgk[:, 0:LQ]), (sgk_b, sgk[:, 0:LQ]),
                               (cgq_b, cgq), (sgq_b, sgq)):
                    nc.sync.dma_start(t[0:64, :], src)
                    nc.sync.dma_start(t[64:128, :], src)
                nc.sync.dma_start(wp_sb[:], wpTr[:])
                nc.sync.dma_start(bp_b[:],
                                  bproj[0:1, :].partition_broadcast(128))

            with tc.tile_pool(name="scp", bufs=2, space="PSUM") as scp, \
                 tc.tile_pool(name="cxp", bufs=2, space="PSUM") as cxp, \
                 tc.tile_pool(name="prj", bufs=2, space="PSUM") as prj:

                hp_state = {}

                # ============== projection steps (dripped) ==============
                def build_proj_steps(hp, per_jc_inv=False):
                    """Closures, each issuing a small slice of head-pair
                    hp's qkv projection; executed interleaved into the
                    previous head-pair's attention instruction stream.
                    per_jc_inv: compute the rms-inv + rope per jc chunk
                    (more DVE ops, but each k/q chunk is ready immediately
                    -- used for hp0 so its attention starts early)."""
                    st = {}
                    hp_state[hp] = st

                    def s_alloc():
                        w = wslp.tile([128, 8, 3, 128], bf16, tag="wsl",
                                      name=f"wsl{hp}")
                        for t in range(3):
                            nc.sync.dma_start(
                                w[:, :, t, :],
                                wTr[:, :, t * C + hp * 128:
                                    t * C + (hp + 1) * 128])
                        st["w"] = w
                        st["qraw"] = trans.tile([128, LQ], bf16, tag="qraw",
                                                name=f"qraw{hp}")
                        st["kraw"] = trans.tile([128, LQ], bf16, tag="kraw",
                                                name=f"kraw{hp}")
                        st["qshf"] = trans.tile([128, LQ], bf16, tag="qshf",
                                                name=f"qshf{hp}")
                        st["kshf"] = trans.tile([128, LQ], bf16, tag="kshf",
                                                name=f"kshf{hp}")
                        st["coll"] = trans.tile([16, 512], bf16, tag="coll",
                                                name=f"coll{hp}")
                        st["collb"] = trans.tile([8, 512], bf16, tag="collb",
                                                 name=f"collb{hp}")
                        st["qhat"] = pairq.tile([128, LQ], bf16, tag="qhat",
                                                name=f"qhat{hp}")
                        st["khat"] = pairk.tile([128, L], bf16, tag="khat",
                                                name=f"khat{hp}")
                        st["vsb"] = pairv.tile([128, 32, 2, 65], bf16,
                                               tag="vsb", name=f"vsb{hp}")
                        nc.vector.memset(st["vsb"][:, :, :, 64:65], 1.0)
                        st["xe"] = {}
                        for eb in range(8):
                            prefetch(0, eb)
                    ALLOC = [s_alloc]

                    def prefetch(jc, eb):
                        t = xep.tile([128, 512], bf16, tag="xe", name="xe")
                        nc.gpsimd.dma_start(t[:], xTr[:, eb, jc * 512:
                                                      (jc + 1) * 512])
                        st["xe"][(jc, eb)] = t

                    def palloc(shape, dtype, name):
                        # during hp0's serial head the attention score pool
                        # is idle -- borrow its banks for a 4-deep
                        # accumulator rotation (single monotone stream, so
                        # no cross-stream pool-order inversion)
                        if st.get("wide"):
                            st["pc"] = st.get("pc", 0) + 1
                            if st["pc"] % 2:
                                return scp.tile(shape, dtype, tag="sc",
                                                name=name)
                        return prj.tile(shape, dtype, tag="pj", name=name)

                    def mk_mm(jc, which, eb, pf):
                        def f():
                            if eb == 0:
                                st["ps"] = palloc([128, 512], f32, "ps")
                            if pf is not None:
                                prefetch(*pf)
                            nc.tensor.matmul(st["ps"][:],
                                             st["w"][:, eb, which, :],
                                             st["xe"][(jc, eb)][:],
                                             start=(eb == 0), stop=(eb == 7))
                        return f

                    def mk_sum(jc, tsr):
                        # rms partial sums + rotate-half shifted copy
                        def f():
                            raw = st["qraw"] if tsr == 0 else st["kraw"]
                            shf = st["qshf"] if tsr == 0 else st["kshf"]
                            sl = slice(jc * 512, (jc + 1) * 512)
                            nc.vector.tensor_copy(raw[:, sl], st["ps"][:])
                            sq = work.tile([128, 512], bf16, tag="sq",
                                           name="sq")
                            nc.vector.tensor_mul(sq[:], raw[:, sl], raw[:, sl])
                            pss = palloc([2, 512], f32, "pss")
                            nc.tensor.matmul(pss[:], onesT[:], sq[:],
                                             start=True, stop=True)
                            cp2 = work.tile([2, 512], bf16, tag="cp2",
                                            name="cp2")
                            nc.vector.tensor_copy(cp2[:], pss[:])
                            if per_jc_inv:
                                st["cp2"] = cp2
                            elif tsr == 0:
                                nc.sync.dma_start(
                                    st["coll"][2 * jc:2 * jc + 2, :], cp2[:])
                            elif jc < 4:
                                nc.sync.dma_start(
                                    st["coll"][8 + 2 * jc:10 + 2 * jc, :],
                                    cp2[:])
                            else:
                                nc.sync.dma_start(
                                    st["collb"][2 * (jc - 4):
                                                2 * (jc - 4) + 2, :], cp2[:])
                            nc.sync.dma_start(shf[0:32, sl], raw[32:64, sl])
                            nc.sync.dma_start(shf[32:64, sl], raw[0:32, sl])
                            nc.sync.dma_start(shf[64:96, sl], raw[96:128, sl])
                            nc.sync.dma_start(shf[96:128, sl], raw[64:96, sl])
                        return f

                    def mk_vcopy(jc):
                        def f():
                            vTc = work.tile([128, 512], bf16, tag="vTc",
                                            name="vTc")
                            nc.vector.tensor_copy(vTc[:], st["ps"][:])
                            st["vTc"] = vTc
                        return f

                    def mk_vtr(jc, jt):
                        def f():
                            jg = jc * 4 + jt
                            pt = palloc([128, 128], bf16, "pt")
                            nc.tensor.transpose(
                                pt[:], st["vTc"][:, jt * 128:(jt + 1) * 128],
                                ident[:])
                            nc.vector.tensor_copy(
                                st["vsb"][:, jg, :, 0:64],
                                pt[:].rearrange("p (h d) -> p h d", h=2))
                        return f

                    def mk_newton2(jc, tsr):
                        # per-chunk rsqrt Newton on [2,512] straight from
                        # the fresh cp2 sums; writes its two inv_sc rows
                        def f():
                            src = st["cp2"]
                            xms = nwt.tile([2, 512], f32, tag="xms",
                                           name="n2x")
                            nc.vector.tensor_scalar(
                                xms[:], src[:], 1.0 / 64.0, EPS,
                                op0=mybir.AluOpType.mult,
                                op1=mybir.AluOpType.add)
                            y = nwt.tile([2, 512], f32, tag="y", name="n2y")
                            nc.vector.tensor_scalar(
                                y[:], xms[:], -RSQ_B, RSQ_A,
                                op0=mybir.AluOpType.mult,
                                op1=mybir.AluOpType.add)
                            t1 = nwt.tile([2, 512], f32, tag="t1",
                                          name="n2t1")
                            t2 = nwt.tile([2, 512], f32, tag="t2",
                                          name="n2t2")
                            inv = nwt.tile([2, 512], bf16, tag="inv",
                                           name="n2i")
                            for it in range(3):
                                nc.vector.tensor_mul(t1[:], xms[:], y[:])
                                nc.vector.tensor_mul(t2[:], t1[:], y[:])
                                nc.vector.tensor_scalar(
                                    t2[:], t2[:], -0.5, 1.5,
                                    op0=mybir.AluOpType.mult,
                                    op1=mybir.AluOpType.add)
                                if it < 2:
                                    nc.vector.tensor_mul(y[:], y[:], t2[:])
                                else:
                                    nc.vector.tensor_mul(inv[:], y[:], t2[:])
                            r0 = 2 * jc if tsr == 0 else 8 + 2 * jc
                            nc.sync.dma_start(inv_sc[hp, r0:r0 + 2, :],
                                              inv[:])
                        return f

                    def jc_steps(jc):
                        s = []
                        for eb in range(8):
                            # prefetch next jc's x a full pass ahead
                            pf = (jc + 1, eb) if jc < 3 else None
                            s.append(mk_mm(jc, 1, eb, pf))
                        s.append(mk_sum(jc, 1))
                        if per_jc_inv:
                            s.append(mk_newton2(jc, 1))
                            s.append(mk_rope(jc, 1))
                        for eb in range(8):
                            s.append(mk_mm(jc, 2, eb, None))
                        s.append(mk_vcopy(jc))
                        for jt in range(4):
                            s.append(mk_vtr(jc, jt))
                        if jc < 4:
                            for eb in range(8):
                                s.append(mk_mm(jc, 0, eb, None))
                            s.append(mk_sum(jc, 0))
                            if per_jc_inv:
                                s.append(mk_newton2(jc, 0))
                                s.append(mk_rope(jc, 0))
                        return s

                    def mk_newton(seg):
                        # rsqrt(ms+eps) via Newton on DVE; seg 0 covers
                        # inv rows 0..15 (q + first-half k), seg 1 rows 16..23
                        def f():
                            src = st["coll"] if seg == 0 else st["collb"]
                            nr = 16 if seg == 0 else 8
                            lo = 0 if seg == 0 else 16
                            xms = nwt.tile([16, 512], f32, tag="xms",
                                           name="xms")[0:nr, :]
                            nc.vector.tensor_scalar(
                                xms[:], src[:], 1.0 / 64.0, EPS,
                                op0=mybir.AluOpType.mult,
                                op1=mybir.AluOpType.add)
                            y = nwt.tile([16, 512], f32, tag="y",
                                         name="y")[0:nr, :]
                            nc.vector.tensor_scalar(
                                y[:], xms[:], -RSQ_B, RSQ_A,
                                op0=mybir.AluOpType.mult,
                                op1=mybir.AluOpType.add)
                            t1 = nwt.tile([16, 512], f32, tag="t1",
                                          name="t1")[0:nr, :]
                            t2 = nwt.tile([16, 512], f32, tag="t2",
                                          name="t2")[0:nr, :]
                            inv = nwt.tile([16, 512], bf16, tag="inv",
                                           name="inv")[0:nr, :]
                            for it in range(3):
                                nc.vector.tensor_mul(t1[:], xms[:], y[:])
                                nc.vector.tensor_mul(t2[:], t1[:], y[:])
                                nc.vector.tensor_scalar(
                                    t2[:], t2[:], -0.5, 1.5,
                                    op0=mybir.AluOpType.mult,
                                    op1=mybir.AluOpType.add)
                                if it < 2:
                                    nc.vector.tensor_mul(y[:], y[:], t2[:])
                                else:
                                    nc.vector.tensor_mul(inv[:], y[:], t2[:])
                            nc.sync.dma_start(inv_sc[hp, lo:lo + nr, :],
                                              inv[:])
                        return f

                    def mk_rope(jc, tsr):
                        # hat = raw*inv*cg + shf*inv*sg  for one 512-chunk
                        def f():
                            if tsr == 0:
                                raw, shf, cg_b, sg_b = (st["qraw"], st["qshf"],
                                                        cgq_b, sgq_b)
                                hat, r0 = st["qhat"], 0
                            else:
                                raw, shf, cg_b, sg_b = (st["kraw"], st["kshf"],
                                                        cgk_b, sgk_b)
                                hat, r0 = st["khat"], 8
                            sl = slice(jc * 512, (jc + 1) * 512)
                            rA = r0 + 2 * jc
                            ib = work.tile([128, 512], bf16, tag="ib",
                                           name="ib")
                            nc.sync.dma_start(
                                ib[0:64, :],
                                inv_sc[hp, rA:rA + 1, :]
                                .partition_broadcast(64))
                            nc.sync.dma_start(
                                ib[64:128, :],
                                inv_sc[hp, rA + 1:rA + 2, :]
                                .partition_broadcast(64))
                            icg = work.tile([128, 512], bf16, tag="icg",
                                            name="icg")
                            nc.vector.tensor_mul(icg[:], ib[:], cg_b[:, sl])
                            isg = work.tile([128, 512], bf16, tag="isg",
                                            name="isg")
                            nc.vector.tensor_mul(isg[:], ib[:], sg_b[:, sl])
                            u = work.tile([128, 512], bf16, tag="u", name="u")
                            nc.vector.tensor_mul(u[:], raw[:, sl], icg[:])
                            v2 = work.tile([128, 512], bf16, tag="v2",
                                           name="v2")
                            nc.vector.tensor_mul(v2[:], shf[:, sl], isg[:])
                            nc.vector.tensor_add(hat[:, sl], u[:], v2[:])
                        return f

                    def mk_exchange():
                        # own khat/vsb halves -> DRAM, AllGather within the
                        # core pair, read back [slot0|slot1].  Odd cores see
                        # keys as [sibling|own] -- softmax is key-order
                        # invariant, so no per-core fixup is needed.
                        def send():
                            nc.sync.dma_start(kex_in[hp, :, 0:LQ],
                                              st["khat"][:, 0:LQ])
                            nc.sync.dma_start(
                                kex_in[hp, :, LQ:4128].rearrange(
                                    "p (j h d) -> p j h d", j=16, h=2),
                                st["vsb"][:, 0:16, :, :])

                        def gather():
                            nc.gpsimd.collective_compute(
                                "AllGather", mybir.AluOpType.bypass,
                                replica_groups=[[0, 1], [2, 3], [4, 5],
                                                [6, 7]],
                                ins=[kex_in[hp, :, :]],
                                outs=[kex_out[hp, :, :, :]])

                        def recv():
                            for s in range(2):
                                nc.sync.dma_start(
                                    st["khat"][:, s * LQ:(s + 1) * LQ],
                                    kex_out[hp, s, :, 0:LQ])
                                nc.sync.dma_start(
                                    st["vsb"][:, s * 16:(s + 1) * 16, :, :],
                                    kex_out[hp, s, :, LQ:4128].rearrange(
                                        "p (j h d) -> p j h d", j=16, h=2))
                        return [send, gather, recv]

                    # own-keys only: k/v/q for jc0-3, one newton batch, then
                    # k-ropes -> launch the exchange -> q-ropes overlap the
                    # link transfer -> read back
                    steps = list(ALLOC)
                    for jc in range(4):
                        steps += jc_steps(jc)
                    steps.append(mk_newton(0))
                    send, gather, recv = mk_exchange()
                    for jc in range(4):
                        steps.append(mk_rope(jc, 1))
                    steps += [send, gather]
                    for jc in range(4):
                        steps.append(mk_rope(jc, 0))
                    steps.append(recv)
                    return steps

                # ============== output projection steps ==============
                def build_outproj_steps(ib_lo, ib_hi):
                    steps = []
                    st = {}

                    def mk_pref(ib):
                        def f():
                            cts = []
                            for cb in range(8):
                                ct = outp.tile([128, 128], bf16, tag="ct",
                                               name="ct", bufs=18)
                                nc.sync.dma_start(
                                    ct[:],
                                    ctx_sc[cb, :, ib * 128:(ib + 1) * 128])
                                cts.append(ct)
                            st[ib] = cts
                        return f

                    def mk_opmm(ib, half, cb):
                        def f():
                            if cb == 0:
                                st["po"] = prj.tile([128, 512], f32, tag="pj",
                                                    name="po")
                            nc.tensor.matmul(
                                st["po"][:], st[ib][cb][:],
                                wp_sb[:, cb, half * 512:(half + 1) * 512],
                                start=(cb == 0), stop=(cb == 7))
                        return f

                    def mk_epi(ib, half):
                        def f():
                            ot = outp.tile([128, 512], f32, tag="ot",
                                           name="ot")
                            nc.vector.tensor_add(
                                ot[:], st["po"][:],
                                bp_b[:, half * 512:(half + 1) * 512])
                            nc.sync.dma_start(
                                out_ap[ib * 128:(ib + 1) * 128,
                                       half * 512:(half + 1) * 512], ot[:])
                        return f

                    def mm_steps(ib):
                        s = []
                        for half in range(2):
                            for cb in range(8):
                                s.append(mk_opmm(ib, half, cb))
                            s.append(mk_epi(ib, half))
                        return s

                    prev = None
                    for ib in range(ib_lo, ib_hi):
                        steps.append(mk_pref(ib))
                        if prev is not None:
                            steps += mm_steps(prev)
                        prev = ib
                    steps += mm_steps(prev)
                    return steps

                class Dripper:
                    """Issues `steps` gradually over slots [lo, hi).  Multiple
                    drippers on one attention pass MUST have disjoint windows:
                    interleaving two projection streams rotates the 2-slot
                    PSUM/xe pools out of program order and deadlocks the
                    in-order engine queues."""

                    def __init__(self, steps, lo, hi):
                        self.steps = steps
                        self.lo = lo
                        self.hi = hi
                        self.i = 0

                    def drip(self, slot):
                        if slot < self.lo:
                            return
                        frac = (slot + 1 - self.lo) / (self.hi - self.lo)
                        tgt = min(len(self.steps),
                                  int(frac * len(self.steps)) + 1)
                        while self.i < tgt:
                            self.steps[self.i]()
                            self.i += 1

                    def finish(self):
                        while self.i < len(self.steps):
                            self.steps[self.i]()
                            self.i += 1

                # =================== attention (flat) ===================
                # 128 iterations per head-pair: 4 query-chunks x 32 key-
                # chunks.  Scores are issued 2 iterations ahead -- across
                # chunk AND head-pair boundaries -- so ACT never waits.
                def issue_sc(g):
                    hp, r = divmod(g, 128)
                    qc, j = divmod(r, 32)
                    st = hp_state[hp]
                    khp, qhp = st["khat"], st["qhat"]
                    q0 = qc * 512
                    sct = scp.tile([128, 1024], f32, tag="sc", name="sct")
                    for h in range(2):
                        nc.tensor.matmul(
                            sct[:, h * 512:(h + 1) * 512],
                            khp[h * 64:(h + 1) * 64, j * 128:(j + 1) * 128],
                            qhp[h * 64:(h + 1) * 64, q0:q0 + 512],
                            start=True, stop=True,
                            tile_position=(h * 64, 0))
                    return sct

                def drain(hp, qc, ctxp):
                    # scale rows 0..63 by 1/row64, bounce to DRAM
                    q0 = qc * 512
                    ctf = [drn.tile([65, 512], bf16, tag=f"ctf{h}",
                                    name=f"ctf{h}") for h in range(2)]
                    for h in range(2):
                        nc.vector.tensor_copy(ctf[h][:], ctxp[h][:])
                    for h in range(2):
                        rcs = drn.tile([1, 512], f32, tag=f"rcs{h}",
                                       name=f"rcs{h}")
                        nc.vector.tensor_copy(rcs[:], ctf[h][64:65, :])
                        rcp = drn.tile([1, 512], f32, tag=f"rcp{h}",
                                       name=f"rcp{h}")
                        nc.vector.reciprocal_approx_fast(out=rcp[:],
                                                         in_=rcs[:])
                        rcpb = drn.tile([1, 512], bf16, tag=f"rcpb{h}",
                                        name=f"rcpb{h}")
                        nc.vector.tensor_copy(rcpb[:], rcp[:])
                        nc.sync.dma_start(rcp_sc[hp, qc, h:h + 1, :],
                                          rcpb[:])
                    for h in range(2):
                        rb = drn.tile([64, 512], bf16, tag=f"rb{h}",
                                      name=f"rb{h}")
                        nc.sync.dma_start(
                            rb[:],
                            rcp_sc[hp, qc, h:h + 1, :].partition_broadcast(64))
                        cto = drn.tile([64, 512], bf16, tag=f"cto{h}",
                                       name=f"cto{h}")
                        nc.vector.tensor_mul(cto[:], ctf[h][0:64, :], rb[:])
                        nc.sync.dma_start(
                            ctx_sc[hp, h * 64:(h + 1) * 64, q0:q0 + 512],
                            cto[:])

                def attention_hp(hp, drippers, pre):
                    # pre: dict g -> sc tile issued by the previous pair
                    st = hp_state[hp]
                    vhp = st["vsb"]
                    g0 = hp * 128
                    sc_t = dict(pre)
                    carry = {}
                    ctxp = None
                    for r in range(128):
                        g = g0 + r
                        qc, j = divmod(r, 32)
                        if j == 0:
                            ctxp = [cxp.tile([65, 512], f32, tag="cx",
                                             name=f"cx{h}")
                                    for h in range(2)]
                        if g not in sc_t:
                            sc_t[g] = issue_sc(g)
                        e = exps.tile([128, 1024], bf16, tag="exps", name="e")
                        nc.scalar.activation(e[:], sc_t.pop(g)[:], AF.Exp,
                                             scale=0.125)
                        for ga in (g + 1, g + 2):
                            if ga in sc_t or ga >= 1024:
                                continue
                            if ga < g0 + 128:
                                sc_t[ga] = issue_sc(ga)
                            else:
                                carry[ga] = issue_sc(ga)
                        for d in drippers:
                            d.drip(r)
                        for h in range(2):
                            nc.tensor.matmul(
                                ctxp[h][:], vhp[:, j, h, :],
                                e[:, h * 512:(h + 1) * 512],
                                start=(j == 0), stop=(j == 31))
                        if j == 31:
                            drain(hp, qc, ctxp)
                    for d in drippers:
                        d.finish()
                    return carry

                # ====================== main loop =======================
                steps0 = build_proj_steps(0)
                hp_state[0]["wide"] = True
                steps0[0]()  # hp0 weight + x DMAs first in the queues
                load_persist()
                for s in steps0[1:]:
                    s()
                hp_state[0]["wide"] = False
                carry = {}
                for hp in range(NPAIR):
                    drippers = []
                    if hp + 1 < NPAIR:
                        # finish by slot 95: the pair-exchange collective at
                        # the stream tail needs link time (and pair-skew
                        # slack) before the next head-pair's attention
                        # reads khat/vsb
                        drippers.append(
                            Dripper(build_proj_steps(hp + 1), 0, 95))
                    else:
                        # last pair: drip the out projection for each query
                        # chunk right after that chunk's ctx drains
                        drippers.append(
                            Dripper(build_outproj_steps(0, 4), 33, 62))
                        drippers.append(
                            Dripper(build_outproj_steps(4, 8), 65, 94))
                        drippers.append(
                            Dripper(build_outproj_steps(8, 12), 97, 126))
                    carry = attention_hp(hp, drippers, carry)

                # remaining output projection (queries 1536..2047)
                for s in build_outproj_steps(12, 16):
                    s()

    nc.compile()
    return nc


def _host_prep(x, W_qkv, q_scale, k_scale, W_proj, b_proj, cos, sin):
    nbf = ml_dtypes.bfloat16
    cosn = np.asarray(cos, np.float32)
    sinn = np.asarray(sin, np.float32)
    qs = np.asarray(q_scale, np.float32)
    ks = np.asarray(k_scale, np.float32)

    def tables(g):
        sign = np.concatenate([-np.ones(D // 2), np.ones(D // 2)]).astype(np.float32)
        gpart = np.concatenate([g[D // 2:], g[:D // 2]])
        cg = cosn * g[None, :]
        sg = sinn * (sign * gpart)[None, :]
        return cg.T.copy(), sg.T.copy()

    cgq_f, sgq_f = tables(qs)
    cgk_f, sgk_f = tables(ks)

    wT = np.asarray(W_qkv, np.float32).T.astype(nbf)
    wpT = np.asarray(W_proj, np.float32).T.astype(nbf)
    bp = np.asarray(b_proj, np.float32).reshape(1, C)
    onesAB = np.zeros((128, 2), nbf)
    onesAB[0:64, 0] = 1.0
    onesAB[64:128, 1] = 1.0

    xn = np.asarray(x, np.float32)
    in_maps = []
    for core in range(NCORES):
        b, qh = core // 2, core % 2
        own = slice(qh * LQ, (qh + 1) * LQ)
        perm = np.r_[np.arange(qh * LQ, (qh + 1) * LQ),
                     np.arange((1 - qh) * LQ, (2 - qh) * LQ)]
        xTc = xn[b].T[:, perm].astype(nbf)
        in_maps.append({
            "xT": np.ascontiguousarray(xTc),
            "wT": wT, "wpT": wpT,
            "cgq": np.ascontiguousarray(cgq_f[:, own]).astype(nbf),
            "sgq": np.ascontiguousarray(sgq_f[:, own]).astype(nbf),
            "cgk": np.ascontiguousarray(cgk_f[:, perm]).astype(nbf),
            "sgk": np.ascontiguousarray(sgk_f[:, perm]).astype(nbf),
            "onesAB": onesAB, "bproj": bp,
        })
    return in_maps


def kernel(x, W_qkv, q_scale, k_scale, W_proj, b_proj, cos, sin, _trace=False):
    global _compiled
    from concourse.bass_utils import run_bass_kernel_spmd
    if _compiled is None:
        _compiled = _build()
    in_maps = _host_prep(x, W_qkv, q_scale, k_scale, W_proj, b_proj, cos, sin)
    res = run_bass_kernel_spmd(_compiled, in_maps, core_ids=list(range(NCORES)),
                               trace=_trace)
    out = np.empty((B, L, C), np.float32)
    for core in range(NCORES):
        b, qh = core // 2, core % 2
        out[b, qh * LQ:(qh + 1) * LQ, :] = res.results[core]["out"]
    kernel._last = res
    return out
